# revision 29
# baseline (speedup 1.0000x reference)
"""Trainium2 Bass kernel for multi-head attention (B=2, L=2048, D=1024, H=16).

Sharding: 8 cores = 2 (batch) x 4 (head-groups of 4 heads).  Each core
computes q/k/v projections for its 4 heads, softmax attention, and two
per-head-pair partial output projections against its 256 rows of W_o.
The host sums the 16 partials (2 per core) into the full output.

Schedule (per core): heads run as two pairs {0,1}, {2,3}.  Each pair's
scores phase is paced by the ScalarE exp stream (the ~1.15us/[128,1024]
throughput floor); the PE fills its slack with projection chains and
segmented PV accumulation (key-tile segments spilled into SBUF fp32
accumulators), so PV pipelines *inside* its own scores phase.  All
inputs arrive as single contiguous DMAs (host pre-packs the layouts);
warm-up matmuls keep the PE clock at 2.4 GHz through the DMA ramp.
"""

import sys

if "/opt/trn_rl_repo" not in sys.path:
    sys.path.insert(0, "/opt/trn_rl_repo")

import numpy as np
import ml_dtypes

import concourse.bass as bass
import concourse.mybir as mybir
import concourse.tile as tile
from concourse import bacc
from concourse.bass_utils import run_bass_kernel_spmd

B, L, D, H = 2, 2048, 1024, 16
HD = D // H          # 64 head dim
NH = 4               # heads per core
GW = NH * HD         # 256 group width
SCALE = (H / D) ** 0.5  # 1/8
P = 128
KT = D // P          # 8 contraction tiles over D
TBLK = L // P        # 16 key blocks of 128
QC = L // 512        # 4 query chunks of 512
BF16 = mybir.dt.bfloat16
F32 = mybir.dt.float32
EXP = mybir.ActivationFunctionType.Exp

PEXP_BUFS = 22       # pp slots: each [128, 2048] bf16 (4KB/partition)
DEBUG_TAPS = False   # add DRAM taps of intermediate tensors


def _build():
    nc = bacc.Bacc(None, target_bir_lowering=False, debug=False)

    # Host-packed layouts (one contiguous DMA each):
    #   x_d:  [128, 2, 8, 1024]  -> partition p, q-half h, D-tile k
    #   w*_d: [128, 8, 256]      -> partition p, D-tile k, head-col c
    #   wo_d: [128, 2, 1024]     -> partition p, pair i, D-col
    x_d = nc.dram_tensor("xp", (P, 2, KT, 1024), BF16, kind="ExternalInput")
    wq_d = nc.dram_tensor("wqp", (P, KT, GW), BF16, kind="ExternalInput")
    wk_d = nc.dram_tensor("wkp", (P, KT, GW), BF16, kind="ExternalInput")
    wv_d = nc.dram_tensor("wvp", (P, KT, GW), BF16, kind="ExternalInput")
    wo_d = nc.dram_tensor("wop", (P, 2, D), BF16, kind="ExternalInput")
    out0_d = nc.dram_tensor("out0", (L, D), BF16, kind="ExternalOutput")
    out1_d = nc.dram_tensor("out1", (L, D), BF16, kind="ExternalOutput")
    outs_d = (out0_d, out1_d)
    if DEBUG_TAPS:
        dbg_q_d = nc.dram_tensor("dbg_q", (P, L), BF16, kind="ExternalOutput")
        dbg_k_d = nc.dram_tensor("dbg_k", (P, L), BF16, kind="ExternalOutput")
        dbg_v_d = nc.dram_tensor("dbg_v", (P, NH * (HD + 1)), BF16, kind="ExternalOutput")
        dbg_pa_d = nc.dram_tensor("dbg_pa", (P, L), BF16, kind="ExternalOutput")
        dbg_pb_d = nc.dram_tensor("dbg_pb", (P, L), BF16, kind="ExternalOutput")
        dbg_acc_d = nc.dram_tensor("dbg_acc", (HD + 1, 512), mybir.dt.float32, kind="ExternalOutput")
        dbg_ao_d = nc.dram_tensor("dbg_ao", (P, L), BF16, kind="ExternalOutput")

    with tile.TileContext(nc) as tc:
        with (
            tc.tile_pool(name="persist", bufs=1) as pers,
            tc.tile_pool(name="pexp", bufs=PEXP_BUFS) as pexp,
            tc.tile_pool(name="rcp", bufs=1) as rcpp,
            tc.tile_pool(name="rbp", bufs=2) as rbp,
            tc.tile_pool(name="osb", bufs=2) as osbp,
            tc.tile_pool(name="spsum", bufs=3, space="PSUM") as sps,
            tc.tile_pool(name="accp", bufs=2, space="PSUM") as accp,
        ):
            # ---- persistent SBUF tensors ----
            x_sb = pers.tile([P, 2 * KT * 1024], BF16, tag="x")
            wq_sb = pers.tile([P, KT * GW], BF16, tag="wq")
            wk_sb = pers.tile([P, KT * GW], BF16, tag="wk")
            wv_sb = pers.tile([P, KT * GW], BF16, tag="wv")
            wo_sb = pers.tile([P, 2 * D], BF16, tag="wo")
            qT = [pers.tile([P, L], BF16, tag=f"qT{m}", name=f"qT{m}") for m in range(2)]
            kTt = [pers.tile([P, L], BF16, tag=f"kT{m}", name=f"kT{m}") for m in range(2)]
            vext = [pers.tile([P, NH * (HD + 1)], BF16, tag=f"vx{t}", name=f"vx{t}") for t in range(TBLK)]
            aoT = [pers.tile([P, L], BF16, tag=f"aoT{m}", name=f"aoT{m}") for m in range(2)]
            # PV accumulators: one per head-in-pair, 4 chunks side by side.
            accs = [pers.tile([HD + 1, L], F32, tag=f"pvacc{i}", name=f"pvacc{i}") for i in range(2)]
            ones64 = pers.tile([1, HD], BF16, tag="ones64")
            warm = pers.tile([1, 2], BF16, tag="warm")

            def xs(k, col, width):
                """Slice of x^T tile k, original columns [col, col+width)."""
                h, l = col // 1024, col % 1024
                base = h * (KT * 1024) + k * 1024 + l
                return x_sb[:, base:base + width]

            # ---- PE warmup: dummy matmuls on (uninitialized) scratch keep
            # HAM busy through the input DMA so real chains start at 2.4 GHz.
            nc.any.memset(ones64[:], 1.0)
            nc.scalar.activation(warm[:], ones64[:, 0:2], EXP)  # exp table load
            wps = sps.tile([P, 1024], F32, tag="sc")
            for i in range(40):
                nc.tensor.matmul(wps[:, 0:512], lhsT=qT[0][:, 0:P],
                                 rhs=qT[0][:, 0:512], start=True, stop=True)

            # ---- input DMA: 7 contiguous transfers, x-half0 first ----
            nc.sync.dma_start(wq_sb[:].rearrange("p (k c) -> p k c", k=KT), wq_d[:, :, :])
            nc.sync.dma_start(x_sb[:, 0:KT * 1024].rearrange("p (k l) -> p k l", k=KT),
                              x_d[:, 0, :, :])
            nc.sync.dma_start(wk_sb[:].rearrange("p (k c) -> p k c", k=KT), wk_d[:, :, :])
            nc.sync.dma_start(x_sb[:, KT * 1024:].rearrange("p (k l) -> p k l", k=KT),
                              x_d[:, 1, :, :])
            nc.sync.dma_start(wv_sb[:].rearrange("p (k c) -> p k c", k=KT), wv_d[:, :, :])
            nc.sync.dma_start(wo_sb[:].rearrange("p (i d) -> p i d", i=2), wo_d[:, :, :])

            # ---- helper emitters ----
            def emit_proj_chain(dst, w_sb_, m, c):
                """dst[m][:, c*512:+512] = (W[m-block] @ x^T)[:, chunk]."""
                ps = accp.tile([P, 512], F32, tag="acc")
                for k in range(KT):
                    nc.tensor.matmul(
                        ps[:],
                        lhsT=w_sb_[:, k * GW + m * P:k * GW + (m + 1) * P],
                        rhs=xs(k, c * 512, 512),
                        start=(k == 0),
                        stop=(k == KT - 1),
                    )
                nc.vector.tensor_copy(dst[m][:, c * 512:(c + 1) * 512], ps[:])

            def emit_v_chain(t):
                """vext[t][:, h*65:h*65+64] = (x @ Wv^T)[t-block] per head; col 64 = 1."""
                ps = accp.tile([P, 512], F32, tag="acc")
                for k in range(KT):
                    nc.tensor.matmul(
                        ps[:, :GW],
                        lhsT=xs(k, t * P, P),
                        rhs=wv_sb[:, k * GW:(k + 1) * GW],
                        start=(k == 0),
                        stop=(k == KT - 1),
                    )
                vv = vext[t][:].rearrange("p (h e) -> p h e", h=NH)
                pv = ps[:, :GW].rearrange("p (h e) -> p h e", h=NH)
                nc.vector.tensor_copy(vv[:, :, 0:HD], pv)
                nc.any.memset(vv[:, :, HD:HD + 1], 1.0)

            pp = {}  # (m, k) -> (ppA, ppB)

            def emit_scores_half(m, k, half):
                """One q-half (1024 cols) of exp(scores) for pair m, key-block
                k.  Strip (0,0) computes head 2m, strip (64,0) head 2m+1."""
                if (m, k) not in pp:
                    pp[(m, k)] = (pexp.tile([P, L], BF16, tag="pp", name=f"pp{m}_{k}a"),
                                  pexp.tile([P, L], BF16, tag="pp", name=f"pp{m}_{k}b"))
                ppA, ppB = pp[(m, k)]
                psA = sps.tile([P, 1024], F32, tag="sc")
                psB = sps.tile([P, 1024], F32, tag="sc")
                for q in range(2):
                    qg = half * 2 + q
                    for strip, ps in ((0, psA), (HD, psB)):
                        nc.tensor.matmul(
                            ps[:, q * 512:(q + 1) * 512],
                            lhsT=kTt[m][strip:strip + HD, k * P:(k + 1) * P],
                            rhs=qT[m][strip:strip + HD, qg * 512:(qg + 1) * 512],
                            start=True,
                            stop=True,
                        )
                nc.scalar.activation(
                    ppA[:, half * 1024:(half + 1) * 1024], psA[:], EXP, scale=SCALE)
                nc.scalar.activation(
                    ppB[:, half * 1024:(half + 1) * 1024], psB[:], EXP, scale=SCALE)

            def scores(m, k):
                emit_scores_half(m, k, 0)
                emit_scores_half(m, k, 1)

            def emit_pv_seg(m, h, c, k0, k1, first):
                """PV key-tiles [k0,k1) for head h (in pair m), q-chunk c:
                accumulate in PSUM, spill-add into the SBUF accumulator."""
                acc = accs[h][:, c * 512:(c + 1) * 512]
                ps = accp.tile([P, 512], F32, tag="acc")
                hg = 2 * m + h
                for k in range(k0, k1):
                    nc.tensor.matmul(
                        ps[0:HD + 1, :],
                        lhsT=vext[k][:, hg * (HD + 1):(hg + 1) * (HD + 1)],
                        rhs=pp[(m, k)][h][:, c * 512:(c + 1) * 512],
                        start=(k == k0),
                        stop=(k == k1 - 1),
                    )
                if first:
                    nc.vector.tensor_copy(acc, ps[0:HD + 1, :])
                else:
                    nc.vector.tensor_add(acc, acc, ps[0:HD + 1, :])

            rbs = {}  # (m, h) -> rrb tile [1, 2048] awaiting fins

            def emit_normprep(m, h):
                """Batched reciprocal of the 4 chunk sum-rows of head h."""
                rr = rcpp.tile([1, L], F32, tag="r")
                nc.vector.tensor_copy(rr[:], accs[h][HD:HD + 1, :])
                nc.vector.reciprocal_approx_fast(rr[:], rr[:])
                rrb = rbp.tile([1, L], BF16, tag="rb")
                if m == 1:
                    nc.scalar.copy(rrb[:], rr[:])
                else:
                    nc.vector.tensor_copy(rrb[:], rr[:])
                rbs[(m, h)] = rrb

            def emit_fin(m, h, c):
                """aoT[m][h-rows, c-chunk] = acc[0:64] * (1/sums) broadcast."""
                rrb = rbs[(m, h)]
                br = accp.tile([P, 512], F32, tag="acc")
                nc.tensor.matmul(br[0:HD, :], lhsT=ones64[:],
                                 rhs=rrb[:, c * 512:(c + 1) * 512],
                                 start=True, stop=True)
                nc.vector.tensor_mul(
                    aoT[m][h * HD:(h + 1) * HD, c * 512:(c + 1) * 512],
                    accs[h][0:HD, c * 512:(c + 1) * 512],
                    br[0:HD, :],
                )

            def emit_oproj2(pair, t0, act_evict=False):
                """outs[pair][t-blocks t0, t0+1] = aoT^T @ woT."""
                for ti in range(2):
                    t = t0 + ti
                    ob = osbp.tile([P, D], BF16, tag="ob")
                    for oc in range(2):
                        ps = accp.tile([P, 512], F32, tag="acc")
                        nc.tensor.matmul(
                            ps[:],
                            lhsT=aoT[pair][:, t * P:(t + 1) * P],
                            rhs=wo_sb[:, pair * D + oc * 512:pair * D + (oc + 1) * 512],
                            start=True,
                            stop=True,
                        )
                        dst = ob[:, oc * 512:(oc + 1) * 512]
                        if act_evict and oc == 0:
                            nc.scalar.copy(dst, ps[:])
                        else:
                            nc.vector.tensor_copy(dst, ps[:])
                    nc.sync.dma_start(outs_d[pair][t * P:(t + 1) * P, :], ob[:])

            # ---- emission schedule ----
            emit_proj_chain(qT, wq_sb, 0, 0)
            emit_proj_chain(qT, wq_sb, 0, 1)
            emit_proj_chain(kTt, wk_sb, 0, 0)

            # S0: half1 of k=0,1 deferred until qT[0] c2/c3 chains are in.
            # segA chains need only vext[0..8) and exp(0,0..7).
            f0 = {
                0: [lambda: emit_proj_chain(qT, wq_sb, 0, 2)],
                1: [lambda: emit_proj_chain(qT, wq_sb, 0, 3),
                    lambda: emit_scores_half(0, 0, 1),
                    lambda: emit_scores_half(0, 1, 1)],
                2: [lambda: emit_proj_chain(kTt, wk_sb, 0, 1), lambda: emit_v_chain(0)],
                3: [lambda: emit_v_chain(1), lambda: emit_v_chain(2)],
                4: [lambda: emit_v_chain(3), lambda: emit_v_chain(4)],
                5: [lambda: emit_v_chain(5), lambda: emit_v_chain(6)],
                6: [lambda: emit_proj_chain(kTt, wk_sb, 0, 2), lambda: emit_v_chain(7)],
                7: [lambda: emit_pv_seg(0, 0, 0, 0, 8, True),
                    lambda: emit_pv_seg(0, 0, 1, 0, 8, True)],
                8: [lambda: emit_pv_seg(0, 0, 2, 0, 8, True),
                    lambda: emit_pv_seg(0, 0, 3, 0, 8, True)],
                9: [lambda: emit_pv_seg(0, 1, 0, 0, 8, True),
                    lambda: emit_pv_seg(0, 1, 1, 0, 8, True)],
                10: [lambda: emit_pv_seg(0, 1, 2, 0, 8, True),
                     lambda: emit_pv_seg(0, 1, 3, 0, 8, True)],
                11: [lambda: emit_proj_chain(kTt, wk_sb, 0, 3),
                     lambda: emit_v_chain(8), lambda: emit_v_chain(9)],
                12: [lambda: emit_proj_chain(qT, wq_sb, 1, 0),
                     lambda: emit_v_chain(10), lambda: emit_v_chain(11)],
                13: [lambda: emit_proj_chain(qT, wq_sb, 1, 1),
                     lambda: emit_v_chain(12), lambda: emit_v_chain(13)],
                14: [lambda: emit_proj_chain(qT, wq_sb, 1, 2),
                     lambda: emit_proj_chain(qT, wq_sb, 1, 3),
                     lambda: emit_v_chain(14), lambda: emit_v_chain(15)],
                15: [lambda: emit_proj_chain(kTt, wk_sb, 1, 0)],
            }
            if DEBUG_TAPS:
                f0[7].insert(0, lambda: (
                    nc.sync.dma_start(dbg_pa_d[:, :], pp[(0, 0)][0][:]),
                    nc.sync.dma_start(dbg_pb_d[:, :], pp[(0, 0)][1][:])))
            for k in range(2):
                emit_scores_half(0, k, 0)
                for f in f0[k]:
                    f()
            for k in range(2, TBLK):
                scores(0, k)
                for f in f0[k]:
                    f()

            # S1: pair-0 segB/segC/norms and out0 projection drain during the
            # pair-1 scores phase; pair-1 segA + a staggered piece of segB too.
            f1 = {
                0: [lambda: emit_pv_seg(0, 0, 0, 8, 14, False),
                    lambda: emit_pv_seg(0, 0, 1, 8, 14, False)],
                1: [lambda: emit_pv_seg(0, 0, 2, 8, 14, False),
                    lambda: emit_pv_seg(0, 0, 3, 8, 14, False),
                    lambda: emit_pv_seg(0, 0, 0, 14, 16, False)],
                2: [lambda: emit_pv_seg(0, 1, 0, 8, 14, False),
                    lambda: emit_pv_seg(0, 1, 1, 8, 14, False),
                    lambda: emit_pv_seg(0, 0, 1, 14, 16, False)],
                3: [lambda: emit_proj_chain(kTt, wk_sb, 1, 1),
                    lambda: emit_pv_seg(0, 1, 2, 8, 14, False),
                    lambda: emit_pv_seg(0, 0, 2, 14, 16, False)],
                4: [lambda: emit_pv_seg(0, 1, 3, 8, 14, False),
                    lambda: emit_pv_seg(0, 0, 3, 14, 16, False)],
                5: [lambda: emit_normprep(0, 0),
                    lambda: emit_pv_seg(0, 1, 0, 14, 16, False),
                    lambda: emit_pv_seg(0, 1, 1, 14, 16, False)],
                6: [lambda: emit_pv_seg(0, 1, 2, 14, 16, False),
                    lambda: emit_pv_seg(0, 1, 3, 14, 16, False),
                    lambda: emit_fin(0, 0, 0)],
                7: [lambda: emit_proj_chain(kTt, wk_sb, 1, 2),
                    lambda: emit_normprep(0, 1),
                    lambda: emit_fin(0, 0, 1), lambda: emit_fin(0, 0, 2)],
                8: [lambda: emit_fin(0, 0, 3),
                    lambda: emit_fin(0, 1, 0), lambda: emit_fin(0, 1, 1),
                    lambda: emit_fin(0, 1, 2), lambda: emit_fin(0, 1, 3)],
                9: [lambda: emit_pv_seg(1, 0, 0, 0, 8, True),
                    lambda: emit_pv_seg(1, 0, 1, 0, 8, True),
                    lambda: emit_pv_seg(1, 0, 2, 0, 8, True),
                    lambda: emit_pv_seg(1, 0, 3, 0, 8, True),
                    lambda: emit_oproj2(0, 0)],
                10: [lambda: emit_pv_seg(1, 1, 0, 0, 8, True),
                     lambda: emit_pv_seg(1, 1, 1, 0, 8, True),
                     lambda: emit_pv_seg(1, 1, 2, 0, 8, True),
                     lambda: emit_pv_seg(1, 1, 3, 0, 8, True),
                     lambda: emit_oproj2(0, 2)],
                11: [lambda: emit_proj_chain(kTt, wk_sb, 1, 3),
                     lambda: emit_oproj2(0, 4)],
                12: [lambda: emit_oproj2(0, 6), lambda: emit_oproj2(0, 8)],
                13: [lambda: emit_pv_seg(1, 0, 0, 8, 12, False),
                     lambda: emit_pv_seg(1, 0, 1, 8, 12, False),
                     lambda: emit_oproj2(0, 10)],
                14: [lambda: emit_pv_seg(1, 0, 2, 8, 12, False),
                     lambda: emit_pv_seg(1, 0, 3, 8, 12, False),
                     lambda: emit_pv_seg(1, 1, 0, 8, 12, False),
                     lambda: emit_oproj2(0, 12)],
                15: [lambda: emit_pv_seg(1, 1, 1, 8, 12, False),
                     lambda: emit_pv_seg(1, 1, 2, 8, 12, False),
                     lambda: emit_pv_seg(1, 1, 3, 8, 12, False)],
            }
            if DEBUG_TAPS:
                f1[0].insert(0, lambda: (
                    nc.sync.dma_start(dbg_q_d[:, :], qT[0][:]),
                    nc.sync.dma_start(dbg_k_d[:, :], kTt[0][:]),
                    nc.sync.dma_start(dbg_v_d[:, :], vext[0][:])))
                f1[7].insert(0, lambda: nc.sync.dma_start(
                    dbg_acc_d[:, :], accs[0][:, 0:512]))
                f1[9].insert(0, lambda: nc.sync.dma_start(dbg_ao_d[:, :], aoT[0][:]))
            for k in range(TBLK):
                scores(1, k)
                for f in f1[k]:
                    f()

            # Tail: last PV segments, pair-1 norms, out1 projection.
            emit_oproj2(0, 14)
            for c in range(QC):
                emit_pv_seg(1, 0, c, 12, 16, False)
            emit_normprep(1, 0)
            for c in range(QC):
                emit_pv_seg(1, 1, c, 12, 16, False)
            emit_normprep(1, 1)
            for c in range(QC):
                emit_fin(1, 0, c)
                emit_fin(1, 1, c)
                emit_oproj2(1, 4 * c, act_evict=True)
                emit_oproj2(1, 4 * c + 2, act_evict=True)
    nc.compile()
    return nc


_NC = None


def _get_nc():
    global _NC
    if _NC is None:
        _NC = _build()
    return _NC


def _shard(inputs):
    x = np.asarray(inputs["x"], dtype=np.float32)
    W_q = np.asarray(inputs["W_q"], dtype=np.float32)
    W_k = np.asarray(inputs["W_k"], dtype=np.float32)
    W_v = np.asarray(inputs["W_v"], dtype=np.float32)
    W_o = np.asarray(inputs["W_o"], dtype=np.float32)
    bf = ml_dtypes.bfloat16
    in_maps = []
    for core in range(8):
        b, g = core // 4, core % 4
        sl = slice(g * GW, (g + 1) * GW)

        def wtiles(w):  # [D, GW] -> [P, KT, GW]: partition-major packed tiles
            return np.ascontiguousarray(
                w.reshape(KT, P, GW).transpose(1, 0, 2)).astype(bf)

        xT = x[b].T  # [D, L]
        xp = np.ascontiguousarray(
            xT.reshape(KT, P, 2, 1024).transpose(1, 2, 0, 3)).astype(bf)
        in_maps.append({
            "xp": xp,
            "wqp": wtiles(W_q[sl, :].T),
            "wkp": wtiles(W_k[sl, :].T),
            "wvp": wtiles(W_v[sl, :].T),
            "wop": np.ascontiguousarray(
                W_o[:, sl].T.reshape(2, P, D).transpose(1, 0, 2)).astype(bf),
        })
    return in_maps


def _run(inputs, trace=False):
    nc = _get_nc()
    in_maps = _shard(inputs)
    res = run_bass_kernel_spmd(nc, in_maps, core_ids=list(range(8)), trace=trace)
    out = np.zeros((B, L, D), dtype=np.float32)
    for core in range(8):
        out[core // 4] += res.results[core]["out0"].astype(np.float32)
        out[core // 4] += res.results[core]["out1"].astype(np.float32)
    return out, res


def kernel(**inputs) -> np.ndarray:
    out, _ = _run(inputs, trace=False)
    return out
